# revision 37
# baseline (speedup 1.0000x reference)
"""GraphSAGE message-passing kernel for 8 Trainium2 NeuronCores.

reference semantics:
    h = relu(feat @ W0)
    deg = segment_sum(ones, dst); inv = 1/max(deg,1)
    for l in 0..2: h = relu((segment_sum(h[src], dst) * inv) @ Ws[l])
    out = concat([h0, h1, h2, h3], axis=1)          # [50000, 512]

Distribution: nodes are greedily bin-packed by in-degree into 392 groups of
<=128 (balanced degree sums), group b owned by core b//49 — a global node
permutation, undone on the host after the run.  Edges live on their dst-owner
core, chunked per (dst-group, src-table-half).  Each layer runs two phases:
phase A gathers lo-half src rows (dma_gather from the replicated bf16 lo
table, 4 SWDGE queues round-robin) and accumulates per-group partial sums
into an SBUF f32 slab; phase B gathers hi-half rows, finishes the segment
sum in PSUM, adds the phase-A partial, scales by inv-degree, applies the
layer weight + ReLU, and emits.  Per-core h chunks are AllGathered into the
next layer's two half-tables; the lo-half AllGather fires mid-phase-B so the
next layer's phase A overlaps the hi-half AllGather.
"""
import sys

sys.path.insert(0, "/opt/trn_rl_repo")

import heapq

import numpy as np
import ml_dtypes

N_NODES = 50000
N_EDGES = 800000
D = 128
NLAYERS = 3
NCORES = 8
NPC = N_NODES // NCORES          # 6250 nodes per core
NGRP = (NPC + 127) // 128        # 49 groups of 128 slots
NPCP = NGRP * 128                # 6272 slots per core
GA = 23                          # groups in half A (lo table)
GB = NGRP - GA                   # 26 groups in half B (hi table)
NA = GA * 128                    # 3200 slots per core in half A
NB = GB * 128                    # 3072 slots in half B
TA = NCORES * NA                 # 25600 lo-table rows
TB = NCORES * NB                 # 24576 hi-table rows
Q1G, Q2G, Q3G, Q4G = 13, 12, 12, 12   # groups per table quarter
Q1N, Q2N = Q1G * 128, Q2G * 128       # rows/core in lo quarters
Q3N, Q4N = Q3G * 128, Q4G * 128       # rows/core in hi quarters
PIECE_G = 2                      # groups gathered/built per pipeline piece

_RUNTIME = {}


def _patch_tile_drain():
    from concourse import mybir
    from concourse.tile import TileContext, ScopedClock

    if getattr(TileContext, "_drain_patched", False):
        return

    def _drain_and_barrier(self, tick_clock, wait_clock):
        # This walrus build rejects >1 sem-wait on one instruction; split the
        # kernel-tail drain waits across single-wait nops on SP.
        nc = self.nc
        probe = nc.sync.nop()
        wait_clock.add_sem_waits(
            probe.ins, ScopedClock({None: tick_clock.global_clock})
        )
        si = probe.ins.sync_info
        waits = list(si.on_wait) if si is not None else []
        if len(waits) > 1:
            si.on_wait = waits[:1]
            for w in waits[1:]:
                n = nc.sync.nop()
                n.ins.sync_info = mybir.SyncInfo(on_wait=[w], on_update=[])
        nc.sync.drain()
        nc.all_engine_barrier()
        popped = nc._tile_sem_poison_stack.pop()
        assert popped is self._sem_poison
        nc.clear_and_free_semaphores(list(self.sems.allocated().values()))
        nc.all_engine_barrier()

    TileContext._drain_and_barrier = _drain_and_barrier
    TileContext._drain_patched = True


def _pack_idxs(idx):
    """Pack one dma_gather call's index sequence.

    Slot L of the gather output sits at partition L%128, free slot L//128;
    the Q7 kernel reads the index for that slot from wrapped[p%16, p//16+8*s].
    Returns [16, n/16] int16 (caller concatenates calls and tiles to 128).
    """
    idx = np.asarray(idx, dtype=np.int16)
    n = len(idx)
    assert n % 128 == 0
    L = np.arange(n)
    s, p = L // 128, L % 128
    wrapped = np.zeros((16, n // 16), dtype=np.int16)
    wrapped[p % 16, p // 16 + 8 * s] = idx
    return wrapped


def _balance_nodes(deg):
    """Greedy bin-pack: nodes into NCORES*NGRP bins of <=128, balancing
    in-degree sums.  Returns pos[node] = global slot id (core*NPCP + ...)."""
    nbins = NCORES * NGRP
    order = np.argsort(-deg, kind="stable")
    heap = [(0.0, b) for b in range(nbins)]
    heapq.heapify(heap)
    counts = np.zeros(nbins, dtype=np.int64)
    binof = np.empty(N_NODES, dtype=np.int64)
    slotof = np.empty(N_NODES, dtype=np.int64)
    spill = []
    for n in order:
        while True:
            s, b = heapq.heappop(heap)
            if counts[b] < 128:
                break
            spill.append((s, b))
        binof[n] = b
        slotof[n] = counts[b]
        counts[b] += 1
        if counts[b] < 128:
            heapq.heappush(heap, (s + deg[n], b))
        for it in spill:
            heapq.heappush(heap, it)
        spill.clear()
    # global slot: core = bin // NGRP, group = bin % NGRP
    core = binof // NGRP
    grp = binof % NGRP
    return core * NPCP + grp * 128 + slotof


def _prepare(feat, src, dst):
    """Host-side balance/sharding/sorting/padding."""
    src = np.asarray(src).astype(np.int64)
    dst = np.asarray(dst).astype(np.int64)
    feat = np.asarray(feat, dtype=np.float32)

    deg = np.bincount(dst, minlength=N_NODES).astype(np.float32)
    invdeg = (1.0 / np.maximum(deg, 1.0)).astype(np.float32)

    pos = _balance_nodes(deg)                     # node -> core*NPCP + p
    owner = pos // NPCP
    p_local = pos - owner * NPCP                  # 0..NPCP-1 (permuted slot)

    sp = pos[src]
    sc, sj = sp // NPCP, sp % NPCP
    hi = sj >= NA                                 # src table half
    srcr = np.where(hi, NB * sc + (sj - NA), NA * sc + sj)
    downer = owner[dst]
    dl = p_local[dst]                             # dst local slot
    group = dl >> 7

    counts = np.zeros((NCORES, NGRP, 2), dtype=np.int64)
    per_core = []
    for c in range(NCORES):
        m = downer == c
        key = (group[m] * 2 + hi[m]).astype(np.int64)
        order = np.argsort(key, kind="stable")
        e_srcr = srcr[m][order]
        e_dstrel = (dl[m] & 127)[order]
        e_key = key[order]
        counts[c] = np.bincount(e_key, minlength=NGRP * 2).reshape(NGRP, 2)
        per_core.append((e_srcr, e_dstrel, e_key))

    # uniform chunk plan: chunks of 128 edges, count = max over cores
    mx = counts.max(axis=0)                       # [NGRP, 2]
    NLO = np.maximum((mx[:, 0] + 127) // 128, 1).astype(np.int64)
    NHI = np.maximum((mx[:, 1] + 127) // 128, 1).astype(np.int64)

    pieces = [
        list(range(p0, min(p0 + PIECE_G, NGRP))) for p0 in range(0, NGRP, PIECE_G)
    ]

    tot_lo = int(NLO.sum()) * 128
    tot_hi = int(NHI.sum()) * 128
    nch_lo = tot_lo // 128
    nch_hi = tot_hi // 128
    totch = nch_lo + nch_hi

    idx_lo = np.zeros((NCORES, 128, tot_lo // 16), dtype=np.int16)
    idx_hi = np.zeros((NCORES, 128, tot_hi // 16), dtype=np.int16)
    # dstrel slab: [all lo chunks group-major | all hi chunks group-major]
    dstrel = np.zeros((NCORES, 128, totch), dtype=ml_dtypes.bfloat16)
    featT = np.zeros((NCORES, D, NPCP), dtype=ml_dtypes.bfloat16)
    invrep = np.zeros((NCORES, 128, NPCP), dtype=ml_dtypes.bfloat16)

    lo_choff = np.concatenate(([0], np.cumsum(NLO)))
    hi_choff = np.concatenate(([0], np.cumsum(NHI)))

    for c in range(NCORES):
        e_srcr, e_dstrel, e_key = per_core[c]
        starts = np.zeros(NGRP * 2 + 1, dtype=np.int64)
        np.cumsum(np.bincount(e_key, minlength=NGRP * 2), out=starts[1:])

        for s, (NC_, choff, base, idxslab) in (
            (0, (NLO, lo_choff, 0, idx_lo)),
            (1, (NHI, hi_choff, nch_lo, idx_hi)),
        ):
            seq = []
            for g in range(NGRP):
                a, b = starts[g * 2 + s], starts[g * 2 + s + 1]
                n_pad = int(NC_[g]) * 128
                ids = np.zeros(n_pad, dtype=np.int64)
                ids[: b - a] = e_srcr[a:b]
                seq.append(ids)
                dr = np.full(n_pad, -1.0, dtype=np.float32)
                dr[: b - a] = e_dstrel[a:b]
                nchk = n_pad // 128
                ch = base + int(choff[g])
                dstrel[c, :, ch : ch + nchk] = (
                    dr.reshape(nchk, 128).T.astype(ml_dtypes.bfloat16)
                )
            seq = np.concatenate(seq)
            blocks = []
            for grp_ in pieces:
                g0, g1 = grp_[0], grp_[-1] + 1
                blocks.append(
                    _pack_idxs(seq[choff[g0] * 128 : choff[g1] * 128])
                )
            idxslab[c] = np.tile(np.concatenate(blocks, axis=1), (8, 1))

    # permuted featT / invdeg
    node_of_pos = np.full(NCORES * NPCP, -1, dtype=np.int64)
    node_of_pos[pos] = np.arange(N_NODES)
    for c in range(NCORES):
        sel = node_of_pos[c * NPCP : (c + 1) * NPCP]
        valid = sel >= 0
        featT[c][:, valid] = feat[sel[valid]].T.astype(ml_dtypes.bfloat16)
        iv = np.zeros(NPCP, dtype=np.float32)
        iv[valid] = invdeg[sel[valid]]
        invrep[c] = np.tile(iv[None, :].astype(ml_dtypes.bfloat16), (128, 1))

    ncnt_lo = np.maximum(((mx[:, 0] + 15) // 16) * 16, 16).astype(np.int64)
    ncnt_hi = np.maximum(((mx[:, 1] + 15) // 16) * 16, 16).astype(np.int64)
    plan = {
        "NLO": NLO.tolist(),
        "NHI": NHI.tolist(),
        "ncnt_lo": ncnt_lo.tolist(),
        "ncnt_hi": ncnt_hi.tolist(),
        "pieces": pieces,
        "tot_lo": tot_lo,
        "tot_hi": tot_hi,
        "nch_lo": nch_lo,
        "nch_hi": nch_hi,
    }
    slabs = {
        "idx_lo": idx_lo,
        "idx_hi": idx_hi,
        "dstrel": dstrel,
        "invrep": invrep,
        "featT": featT,
        "pos": pos,
    }
    return plan, slabs


def _build(plan, ablate=()):
    from concourse import mybir, tile, bacc

    _patch_tile_drain()

    NLO, NHI = plan["NLO"], plan["NHI"]
    ncnt_lo, ncnt_hi = plan["ncnt_lo"], plan["ncnt_hi"]
    pieces = plan["pieces"]
    tot_lo, tot_hi = plan["tot_lo"], plan["tot_hi"]
    nch_lo, nch_hi = plan["nch_lo"], plan["nch_hi"]
    totch = nch_lo + nch_hi
    bf16, f32, i16 = mybir.dt.bfloat16, mybir.dt.float32, mybir.dt.int16

    nc = bacc.Bacc("TRN2", num_swdge_queues=4)
    p_idx_lo = nc.declare_dram_parameter("idx_lo", [128, tot_lo // 16], i16, isOutput=False)
    p_idx_hi = nc.declare_dram_parameter("idx_hi", [128, tot_hi // 16], i16, isOutput=False)
    p_dstrel = nc.declare_dram_parameter("dstrel", [128, totch], bf16, isOutput=False)
    p_invrep = nc.declare_dram_parameter("invrep", [128, NPCP], bf16, isOutput=False)
    p_featT = nc.declare_dram_parameter("featT", [D, NPCP], bf16, isOutput=False)
    p_W0 = nc.declare_dram_parameter("W0", [D, D], bf16, isOutput=False)
    p_Ws = nc.declare_dram_parameter("Ws", [D, NLAYERS, D], bf16, isOutput=False)
    p_out = nc.declare_dram_parameter("out", [NPCP, (NLAYERS + 1) * D], f32, isOutput=True)

    iota_np = np.tile(np.arange(128, dtype=ml_dtypes.bfloat16)[None, :], (128, 1))
    eye_np = np.eye(128, dtype=ml_dtypes.bfloat16)

    lo_choff = np.concatenate(([0], np.cumsum(NLO)))
    hi_choff = np.concatenate(([0], np.cumsum(NHI)))

    with tile.TileContext(nc) as tc:
        iota_d = nc.inline_tensor(iota_np, name="iota_c")
        eye_d = nc.inline_tensor(eye_np, name="eye_c")
        with (
            tc.tile_pool(name="const", bufs=1) as cpool,
            tc.tile_pool(name="glo", bufs=10) as glo_pool,
            tc.tile_pool(name="ghi", bufs=10) as ghi_pool,
            tc.tile_pool(name="sel", bufs=4) as sel_pool,
            tc.tile_pool(name="lacc", bufs=2) as lacc_pool,
            tc.tile_pool(name="small", bufs=6) as small,
            tc.tile_pool(name="psA", bufs=4, space="PSUM") as psA,
            tc.tile_pool(name="psC", bufs=4, space="PSUM") as psC,
            tc.tile_pool(name="dram", bufs=1, space="DRAM") as dram,
        ):
            featT0 = None
            W0 = cpool.tile([D, D], bf16)
            nc.sync.dma_start(out=W0[:], in_=p_W0[:, :])
            iota = cpool.tile([128, 128], bf16)
            nc.sync.dma_start(out=iota[:], in_=iota_d[:, :])
            eye = cpool.tile([128, 128], bf16)
            nc.sync.dma_start(out=eye[:], in_=eye_d[:, :])
            Ws = cpool.tile([D, NLAYERS, D], bf16)
            nc.sync.dma_start(out=Ws[:], in_=p_Ws[:, :, :])
            invrep = cpool.tile([128, NPCP], bf16)
            nc.sync.dma_start(out=invrep[:], in_=p_invrep[:, :])
            featT = cpool.tile([D, NPCP], bf16)
            nc.sync.dma_start(out=featT[:], in_=p_featT[:, :])
            idxlo = cpool.tile([128, tot_lo // 16], i16)
            nc.sync.dma_start(out=idxlo[:], in_=p_idx_lo[:, :])
            idxhi = cpool.tile([128, tot_hi // 16], i16)
            nc.sync.dma_start(out=idxhi[:], in_=p_idx_hi[:, :])
            dstrel = cpool.tile([128, totch], bf16)
            nc.sync.dma_start(out=dstrel[:], in_=p_dstrel[:, :])

            ag_a = [
                dram.tile([NA, D], bf16, tag=f"aga{i}", name=f"aga{i}")
                for i in range(NLAYERS)
            ]
            ag_b = [
                dram.tile([NB, D], bf16, tag=f"agb{i}", name=f"agb{i}")
                for i in range(NLAYERS)
            ]
            tab_lo = [
                dram.tile(
                    [TA, D], bf16, addr_space="Shared",
                    tag=f"tlo{i}", name=f"tlo{i}",
                )
                for i in range(NLAYERS)
            ]
            tab_hi = [
                dram.tile(
                    [TB, D], bf16, addr_space="Shared",
                    tag=f"thi{i}", name=f"thi{i}",
                )
                for i in range(NLAYERS)
            ]

            def ag_quarter(layer, q):
                if "cc" in ablate:
                    return
                if q == 0 and "cca" in ablate:
                    return
                if q == 1 and "ccb" in ablate:
                    return
                if q == 0:
                    src_ap = ag_a[layer][:]
                    dst_ap = tab_lo[layer][0:TA, :]
                else:
                    src_ap = ag_b[layer][:]
                    dst_ap = tab_hi[layer][0:TB, :]
                nc.gpsimd.collective_compute(
                    "AllGather", mybir.AluOpType.bypass,
                    replica_groups=[list(range(NCORES))],
                    ins=[src_ap], outs=[dst_ap],
                )

            def emit_h_block(b, h_ps, layer):
                """Evacuate one [128 nodes, 128] psum block: relu -> out cols,
                bf16 copy -> ag buffers (except last layer)."""
                r0 = b * 128
                h_f = small.tile([128, D], f32, tag="hf")
                nc.scalar.activation(
                    out=h_f[:], in_=h_ps[:],
                    func=mybir.ActivationFunctionType.Relu,
                )
                if "out" not in ablate:
                    nc.sync.dma_start(
                        out=p_out[r0 : r0 + 128, layer * D : (layer + 1) * D],
                        in_=h_f[:],
                    )
                if layer < NLAYERS:
                    h_b = small.tile([128, D], bf16, tag="hb")
                    nc.scalar.activation(
                        out=h_b[:], in_=h_ps[:],
                        func=mybir.ActivationFunctionType.Relu,
                    )
                    if b < GA:
                        nc.sync.dma_start(
                            out=ag_a[layer][r0 : r0 + 128, :], in_=h_b[:]
                        )
                    else:
                        rb = (b - GA) * 128
                        nc.sync.dma_start(
                            out=ag_b[layer][rb : rb + 128, :], in_=h_b[:]
                        )
                    if b == GA - 1:
                        ag_quarter(layer, 0)
                    elif b == NGRP - 1:
                        ag_quarter(layer, 1)

            def build_S(choff_base, c0, c1):
                """One-hot select matrix for chunk columns [c0, c1)."""
                nch = c1 - c0
                S = sel_pool.tile([128, nch, 128], bf16, tag="sel")
                if "sbuild" in ablate:
                    nc.vector.memset(S[:, 0:1, :], 0)
                else:
                    nc.vector.tensor_tensor(
                        out=S[:],
                        in0=dstrel[:, choff_base + c0 : choff_base + c1][:, :, None]
                        .to_broadcast([128, nch, 128]),
                        in1=iota[:][:, None, :].to_broadcast([128, nch, 128]),
                        op=mybir.AluOpType.is_equal,
                    )
                return S

            # ---- phase 0: h0 = relu(feat @ W0) ----
            for b in range(NGRP):
                h_ps = psC.tile([128, D], f32, space="PSUM", tag="hps")
                nc.tensor.matmul(
                    out=h_ps[:], lhsT=featT[:, b * 128 : (b + 1) * 128],
                    rhs=W0[:], start=True, stop=True,
                )
                emit_h_block(b, h_ps, 0)

            # ---- layers: phase A (lo chunks -> loacc), phase B (hi) ----
            for l in range(NLAYERS):
                loacc = lacc_pool.tile([128, NPCP], bf16, tag="loacc")
                for pi, grp in enumerate(pieces):
                    g0, g1 = grp[0], grp[-1] + 1
                    plo = int(lo_choff[g1] - lo_choff[g0])
                    G_lo = glo_pool.tile([128, plo, D], bf16, tag="glo")
                    if "gather" in ablate:
                        nc.vector.memset(G_lo[:, :, 0:1], 0)
                    else:
                        if l == 0 and pi < 10:
                            # first pool-slot use: zero the tail slots that
                            # truncated gathers leave unwritten (0*NaN poisons
                            # PSUM even under a zero S column)
                            nc.vector.memset(G_lo[:], 0)
                        for gi, g in enumerate(grp):
                            co = int(lo_choff[g] - lo_choff[g0])
                            nw = int(lo_choff[g + 1] - lo_choff[g])
                            nc.gpsimd.dma_gather(
                                G_lo[:, co : co + nw, :], tab_lo[l][:, :],
                                idxlo[:, lo_choff[g] * 8 : lo_choff[g + 1] * 8],
                                int(ncnt_lo[g]), int(ncnt_lo[g]), D,
                                single_packet=False,
                                queue_num=(2 * pi + gi) % 4,
                            )
                    S = build_S(0, int(lo_choff[g0]), int(lo_choff[g1]))
                    for g in grp:
                        acc_ps = psA.tile([128, D], f32, space="PSUM", tag="accT")
                        nmm = NLO[g] if "seg" not in ablate else 1
                        for j in range(nmm):
                            nc.tensor.matmul(
                                out=acc_ps[:],
                                lhsT=G_lo[:, int(lo_choff[g] - lo_choff[g0]) + j, :],
                                rhs=S[:, int(lo_choff[g] - lo_choff[g0]) + j, :],
                                start=(j == 0), stop=(j == nmm - 1),
                            )
                        nc.scalar.activation(
                            out=loacc[:, g * 128 : (g + 1) * 128], in_=acc_ps[:],
                            func=mybir.ActivationFunctionType.Copy,
                        )
                for pi, grp in enumerate(pieces):
                    g0, g1 = grp[0], grp[-1] + 1
                    phi = int(hi_choff[g1] - hi_choff[g0])
                    G_hi = ghi_pool.tile([128, phi, D], bf16, tag="ghi")
                    if "gather" in ablate:
                        nc.vector.memset(G_hi[:, :, 0:1], 0)
                    else:
                        if l == 0 and pi < 10:
                            nc.vector.memset(G_hi[:], 0)
                        for gi, g in enumerate(grp):
                            co = int(hi_choff[g] - hi_choff[g0])
                            nw = int(hi_choff[g + 1] - hi_choff[g])
                            nc.gpsimd.dma_gather(
                                G_hi[:, co : co + nw, :], tab_hi[l][:, :],
                                idxhi[:, hi_choff[g] * 8 : hi_choff[g + 1] * 8],
                                int(ncnt_hi[g]), int(ncnt_hi[g]), D,
                                single_packet=False,
                                queue_num=(2 * pi + gi) % 4,
                            )
                    S = build_S(nch_lo, int(hi_choff[g0]), int(hi_choff[g1]))
                    for g in grp:
                        acc_ps = psA.tile([128, D], f32, space="PSUM", tag="accT")
                        # inject the phase-A partial via an identity matmul so
                        # the add rides the PSUM accumulation chain (PE idle,
                        # DVE loaded)
                        nc.tensor.matmul(
                            out=acc_ps[:], lhsT=eye[:],
                            rhs=loacc[:, g * 128 : (g + 1) * 128],
                            start=True, stop=False,
                        )
                        nmm = NHI[g] if "seg" not in ablate else 1
                        for j in range(nmm):
                            nc.tensor.matmul(
                                out=acc_ps[:],
                                lhsT=G_hi[:, int(hi_choff[g] - hi_choff[g0]) + j, :],
                                rhs=S[:, int(hi_choff[g] - hi_choff[g0]) + j, :],
                                start=False, stop=(j == nmm - 1),
                            )
                        aggT = small.tile([128, D], bf16, tag="aggTsb")
                        nc.vector.tensor_tensor(
                            out=aggT[:], in0=acc_ps[:],
                            in1=invrep[:, g * 128 : (g + 1) * 128],
                            op=mybir.AluOpType.mult,
                        )
                        h_ps = psC.tile([128, D], f32, space="PSUM", tag="hps")
                        nc.tensor.matmul(
                            out=h_ps[:], lhsT=aggT[:], rhs=Ws[:, l, :],
                            start=True, stop=True,
                        )
                        emit_h_block(g, h_ps, l + 1)
    nc.compile()
    return nc


def kernel(feat, src, dst, W0, Ws):
    from concourse.bass_utils import run_bass_kernel_spmd

    plan, slabs = _prepare(feat, src, dst)
    nc = _build(plan)

    W0_np = np.asarray(W0, dtype=np.float32).astype(ml_dtypes.bfloat16)
    Ws_np = (
        np.transpose(np.asarray(Ws, dtype=np.float32), (1, 0, 2))
        .astype(ml_dtypes.bfloat16)
    )  # [fi, layer, fo]
    in_maps = [
        {
            "idx_lo": slabs["idx_lo"][c],
            "idx_hi": slabs["idx_hi"][c],
            "dstrel": slabs["dstrel"][c],
            "invrep": slabs["invrep"][c],
            "featT": slabs["featT"][c],
            "W0": W0_np,
            "Ws": Ws_np,
        }
        for c in range(NCORES)
    ]
    res = None
    last_err = None
    for attempt in range(3):
        try:
            res = run_bass_kernel_spmd(nc, in_maps, core_ids=list(range(NCORES)))
            break
        except Exception as e:  # transient device hiccups (axon RPC, NRT recovery)
            last_err = e
            import time as _time

            _time.sleep(5)
    if res is None:
        raise last_err
    _RUNTIME["nc"] = nc
    _RUNTIME["in_maps"] = in_maps

    pos = slabs["pos"]
    allrows = np.concatenate(
        [res.results[c]["out"] for c in range(NCORES)], axis=0
    )  # [NCORES*NPCP, 512]
    return allrows[pos].astype(np.float32)


# revision 38
# speedup vs baseline: 1.0474x; 1.0474x over previous
"""GraphSAGE message-passing kernel for 8 Trainium2 NeuronCores.

reference semantics:
    h = relu(feat @ W0)
    deg = segment_sum(ones, dst); inv = 1/max(deg,1)
    for l in 0..2: h = relu((segment_sum(h[src], dst) * inv) @ Ws[l])
    out = concat([h0, h1, h2, h3], axis=1)          # [50000, 512]

Distribution: nodes are greedily bin-packed by in-degree into 392 groups of
<=128 (balanced degree sums), group b owned by core b//49 — a global node
permutation, undone on the host after the run.  Edges live on their dst-owner
core, chunked per (dst-group, src-table-half).  Each layer runs two phases:
phase A gathers lo-half src rows (dma_gather from the replicated bf16 lo
table, 4 SWDGE queues round-robin) and accumulates per-group partial sums
into an SBUF f32 slab; phase B gathers hi-half rows, finishes the segment
sum in PSUM, adds the phase-A partial, scales by inv-degree, applies the
layer weight + ReLU, and emits.  Per-core h chunks are AllGathered into the
next layer's two half-tables; the lo-half AllGather fires mid-phase-B so the
next layer's phase A overlaps the hi-half AllGather.
"""
import sys

sys.path.insert(0, "/opt/trn_rl_repo")

import heapq

import numpy as np
import ml_dtypes

N_NODES = 50000
N_EDGES = 800000
D = 128
NLAYERS = 3
NCORES = 8
NPC = N_NODES // NCORES          # 6250 nodes per core
NGRP = (NPC + 127) // 128        # 49 groups of 128 slots
NPCP = NGRP * 128                # 6272 slots per core
GA = 23                          # groups in half A (lo table)
GB = NGRP - GA                   # 26 groups in half B (hi table)
NA = GA * 128                    # 3200 slots per core in half A
NB = GB * 128                    # 3072 slots in half B
TA = NCORES * NA                 # 25600 lo-table rows
TB = NCORES * NB                 # 24576 hi-table rows
Q1G, Q2G, Q3G, Q4G = 13, 12, 12, 12   # groups per table quarter
Q1N, Q2N = Q1G * 128, Q2G * 128       # rows/core in lo quarters
Q3N, Q4N = Q3G * 128, Q4G * 128       # rows/core in hi quarters
PIECE_G = 2                      # groups gathered/built per pipeline piece

_RUNTIME = {}


def _patch_tile_drain():
    from concourse import mybir
    from concourse.tile import TileContext, ScopedClock

    if getattr(TileContext, "_drain_patched", False):
        return

    def _drain_and_barrier(self, tick_clock, wait_clock):
        # This walrus build rejects >1 sem-wait on one instruction; split the
        # kernel-tail drain waits across single-wait nops on SP.
        nc = self.nc
        probe = nc.sync.nop()
        wait_clock.add_sem_waits(
            probe.ins, ScopedClock({None: tick_clock.global_clock})
        )
        si = probe.ins.sync_info
        waits = list(si.on_wait) if si is not None else []
        if len(waits) > 1:
            si.on_wait = waits[:1]
            for w in waits[1:]:
                n = nc.sync.nop()
                n.ins.sync_info = mybir.SyncInfo(on_wait=[w], on_update=[])
        nc.sync.drain()
        nc.all_engine_barrier()
        popped = nc._tile_sem_poison_stack.pop()
        assert popped is self._sem_poison
        nc.clear_and_free_semaphores(list(self.sems.allocated().values()))
        nc.all_engine_barrier()

    TileContext._drain_and_barrier = _drain_and_barrier
    TileContext._drain_patched = True


def _pack_idxs(idx):
    """Pack one dma_gather call's index sequence.

    Slot L of the gather output sits at partition L%128, free slot L//128;
    the Q7 kernel reads the index for that slot from wrapped[p%16, p//16+8*s].
    Returns [16, n/16] int16 (caller concatenates calls and tiles to 128).
    """
    idx = np.asarray(idx, dtype=np.int16)
    n = len(idx)
    assert n % 128 == 0
    L = np.arange(n)
    s, p = L // 128, L % 128
    wrapped = np.zeros((16, n // 16), dtype=np.int16)
    wrapped[p % 16, p // 16 + 8 * s] = idx
    return wrapped


def _balance_nodes(deg):
    """Greedy bin-pack: nodes into NCORES*NGRP bins of <=128, balancing
    in-degree sums.  Returns pos[node] = global slot id (core*NPCP + ...)."""
    nbins = NCORES * NGRP
    order = np.argsort(-deg, kind="stable")
    heap = [(0.0, b) for b in range(nbins)]
    heapq.heapify(heap)
    counts = np.zeros(nbins, dtype=np.int64)
    binof = np.empty(N_NODES, dtype=np.int64)
    slotof = np.empty(N_NODES, dtype=np.int64)
    spill = []
    for n in order:
        while True:
            s, b = heapq.heappop(heap)
            if counts[b] < 128:
                break
            spill.append((s, b))
        binof[n] = b
        slotof[n] = counts[b]
        counts[b] += 1
        if counts[b] < 128:
            heapq.heappush(heap, (s + deg[n], b))
        for it in spill:
            heapq.heappush(heap, it)
        spill.clear()
    # global slot: core = bin // NGRP, group = bin % NGRP
    core = binof // NGRP
    grp = binof % NGRP
    return core * NPCP + grp * 128 + slotof


def _prepare(feat, src, dst):
    """Host-side balance/sharding/sorting/padding."""
    src = np.asarray(src).astype(np.int64)
    dst = np.asarray(dst).astype(np.int64)
    feat = np.asarray(feat, dtype=np.float32)

    deg = np.bincount(dst, minlength=N_NODES).astype(np.float32)
    invdeg = (1.0 / np.maximum(deg, 1.0)).astype(np.float32)

    pos = _balance_nodes(deg)                     # node -> core*NPCP + p
    owner = pos // NPCP
    p_local = pos - owner * NPCP                  # 0..NPCP-1 (permuted slot)

    sp = pos[src]
    sc, sj = sp // NPCP, sp % NPCP
    hi = sj >= NA                                 # src table half
    srcr = np.where(hi, NB * sc + (sj - NA), NA * sc + sj)
    downer = owner[dst]
    dl = p_local[dst]                             # dst local slot
    group = dl >> 7

    counts = np.zeros((NCORES, NGRP, 2), dtype=np.int64)
    per_core = []
    for c in range(NCORES):
        m = downer == c
        key = (group[m] * 2 + hi[m]).astype(np.int64)
        order = np.argsort(key, kind="stable")
        e_srcr = srcr[m][order]
        e_dstrel = (dl[m] & 127)[order]
        e_key = key[order]
        counts[c] = np.bincount(e_key, minlength=NGRP * 2).reshape(NGRP, 2)
        per_core.append((e_srcr, e_dstrel, e_key))

    # uniform chunk plan: chunks of 128 edges, count = max over cores
    mx = counts.max(axis=0)                       # [NGRP, 2]
    NLO = np.maximum((mx[:, 0] + 127) // 128, 1).astype(np.int64)
    NHI = np.maximum((mx[:, 1] + 127) // 128, 1).astype(np.int64)

    pieces = [
        list(range(p0, min(p0 + PIECE_G, NGRP))) for p0 in range(0, NGRP, PIECE_G)
    ]

    tot_lo = int(NLO.sum()) * 128
    tot_hi = int(NHI.sum()) * 128
    nch_lo = tot_lo // 128
    nch_hi = tot_hi // 128
    totch = nch_lo + nch_hi

    idx_lo = np.zeros((NCORES, 128, tot_lo // 16), dtype=np.int16)
    idx_hi = np.zeros((NCORES, 128, tot_hi // 16), dtype=np.int16)
    # dstrel slab: [all lo chunks group-major | all hi chunks group-major]
    dstrel = np.zeros((NCORES, 128, totch), dtype=ml_dtypes.bfloat16)
    featT = np.zeros((NCORES, D, NPCP), dtype=ml_dtypes.bfloat16)
    invrep = np.zeros((NCORES, 128, NPCP), dtype=ml_dtypes.bfloat16)

    lo_choff = np.concatenate(([0], np.cumsum(NLO)))
    hi_choff = np.concatenate(([0], np.cumsum(NHI)))

    for c in range(NCORES):
        e_srcr, e_dstrel, e_key = per_core[c]
        starts = np.zeros(NGRP * 2 + 1, dtype=np.int64)
        np.cumsum(np.bincount(e_key, minlength=NGRP * 2), out=starts[1:])

        for s, (NC_, choff, base, idxslab) in (
            (0, (NLO, lo_choff, 0, idx_lo)),
            (1, (NHI, hi_choff, nch_lo, idx_hi)),
        ):
            seq = []
            for g in range(NGRP):
                a, b = starts[g * 2 + s], starts[g * 2 + s + 1]
                n_pad = int(NC_[g]) * 128
                ids = np.zeros(n_pad, dtype=np.int64)
                ids[: b - a] = e_srcr[a:b]
                seq.append(ids)
                dr = np.full(n_pad, -1.0, dtype=np.float32)
                dr[: b - a] = e_dstrel[a:b]
                nchk = n_pad // 128
                ch = base + int(choff[g])
                dstrel[c, :, ch : ch + nchk] = (
                    dr.reshape(nchk, 128).T.astype(ml_dtypes.bfloat16)
                )
            seq = np.concatenate(seq)
            blocks = []
            for grp_ in pieces:
                g0, g1 = grp_[0], grp_[-1] + 1
                blocks.append(
                    _pack_idxs(seq[choff[g0] * 128 : choff[g1] * 128])
                )
            idxslab[c] = np.tile(np.concatenate(blocks, axis=1), (8, 1))

    # permuted featT / invdeg
    node_of_pos = np.full(NCORES * NPCP, -1, dtype=np.int64)
    node_of_pos[pos] = np.arange(N_NODES)
    for c in range(NCORES):
        sel = node_of_pos[c * NPCP : (c + 1) * NPCP]
        valid = sel >= 0
        featT[c][:, valid] = feat[sel[valid]].T.astype(ml_dtypes.bfloat16)
        iv = np.zeros(NPCP, dtype=np.float32)
        iv[valid] = invdeg[sel[valid]]
        invrep[c] = np.tile(iv[None, :].astype(ml_dtypes.bfloat16), (128, 1))

    ncnt_lo = np.maximum(((mx[:, 0] + 15) // 16) * 16, 16).astype(np.int64)
    ncnt_hi = np.maximum(((mx[:, 1] + 15) // 16) * 16, 16).astype(np.int64)
    plan = {
        "NLO": NLO.tolist(),
        "NHI": NHI.tolist(),
        "ncnt_lo": ncnt_lo.tolist(),
        "ncnt_hi": ncnt_hi.tolist(),
        "pieces": pieces,
        "tot_lo": tot_lo,
        "tot_hi": tot_hi,
        "nch_lo": nch_lo,
        "nch_hi": nch_hi,
    }
    slabs = {
        "idx_lo": idx_lo,
        "idx_hi": idx_hi,
        "dstrel": dstrel,
        "invrep": invrep,
        "featT": featT,
        "pos": pos,
    }
    return plan, slabs


def _build(plan, ablate=()):
    from concourse import mybir, tile, bacc

    _patch_tile_drain()

    NLO, NHI = plan["NLO"], plan["NHI"]
    ncnt_lo, ncnt_hi = plan["ncnt_lo"], plan["ncnt_hi"]
    pieces = plan["pieces"]
    tot_lo, tot_hi = plan["tot_lo"], plan["tot_hi"]
    nch_lo, nch_hi = plan["nch_lo"], plan["nch_hi"]
    totch = nch_lo + nch_hi
    bf16, f32, i16 = mybir.dt.bfloat16, mybir.dt.float32, mybir.dt.int16

    nc = bacc.Bacc("TRN2", num_swdge_queues=4)
    p_idx_lo = nc.declare_dram_parameter("idx_lo", [128, tot_lo // 16], i16, isOutput=False)
    p_idx_hi = nc.declare_dram_parameter("idx_hi", [128, tot_hi // 16], i16, isOutput=False)
    p_dstrel = nc.declare_dram_parameter("dstrel", [128, totch], bf16, isOutput=False)
    p_invrep = nc.declare_dram_parameter("invrep", [128, NPCP], bf16, isOutput=False)
    p_featT = nc.declare_dram_parameter("featT", [D, NPCP], bf16, isOutput=False)
    p_W0 = nc.declare_dram_parameter("W0", [D, D], bf16, isOutput=False)
    p_Ws = nc.declare_dram_parameter("Ws", [D, NLAYERS, D], bf16, isOutput=False)
    p_out = nc.declare_dram_parameter("out", [NPCP, (NLAYERS + 1) * D], f32, isOutput=True)

    iota_np = np.tile(np.arange(128, dtype=ml_dtypes.bfloat16)[None, :], (128, 1))
    eye_np = np.eye(128, dtype=ml_dtypes.bfloat16)

    lo_choff = np.concatenate(([0], np.cumsum(NLO)))
    hi_choff = np.concatenate(([0], np.cumsum(NHI)))

    with tile.TileContext(nc) as tc:
        iota_d = nc.inline_tensor(iota_np, name="iota_c")
        eye_d = nc.inline_tensor(eye_np, name="eye_c")
        with (
            tc.tile_pool(name="const", bufs=1) as cpool,
            tc.tile_pool(name="glo", bufs=12) as glo_pool,
            tc.tile_pool(name="ghi", bufs=12) as ghi_pool,
            tc.tile_pool(name="sel", bufs=4) as sel_pool,
            tc.tile_pool(name="lacc", bufs=2) as lacc_pool,
            tc.tile_pool(name="small", bufs=6) as small,
            tc.tile_pool(name="psA", bufs=4, space="PSUM") as psA,
            tc.tile_pool(name="psC", bufs=4, space="PSUM") as psC,
            tc.tile_pool(name="dram", bufs=1, space="DRAM") as dram,
        ):
            featT0 = None
            W0 = cpool.tile([D, D], bf16)
            nc.sync.dma_start(out=W0[:], in_=p_W0[:, :])
            iota = cpool.tile([128, 128], bf16)
            nc.sync.dma_start(out=iota[:], in_=iota_d[:, :])
            eye = cpool.tile([128, 128], bf16)
            nc.sync.dma_start(out=eye[:], in_=eye_d[:, :])
            Ws = cpool.tile([D, NLAYERS, D], bf16)
            nc.sync.dma_start(out=Ws[:], in_=p_Ws[:, :, :])
            invrep = cpool.tile([128, NPCP], bf16)
            nc.sync.dma_start(out=invrep[:], in_=p_invrep[:, :])
            featT = cpool.tile([D, NPCP], bf16)
            nc.sync.dma_start(out=featT[:], in_=p_featT[:, :])
            idxlo = cpool.tile([128, tot_lo // 16], i16)
            nc.sync.dma_start(out=idxlo[:], in_=p_idx_lo[:, :])
            idxhi = cpool.tile([128, tot_hi // 16], i16)
            nc.sync.dma_start(out=idxhi[:], in_=p_idx_hi[:, :])
            dstrel = cpool.tile([128, totch], bf16)
            nc.sync.dma_start(out=dstrel[:], in_=p_dstrel[:, :])

            ag_a = [
                dram.tile([NA, D], bf16, tag=f"aga{i}", name=f"aga{i}")
                for i in range(NLAYERS)
            ]
            ag_b = [
                dram.tile([NB, D], bf16, tag=f"agb{i}", name=f"agb{i}")
                for i in range(NLAYERS)
            ]
            tab_lo = [
                dram.tile(
                    [TA, D], bf16, addr_space="Shared",
                    tag=f"tlo{i}", name=f"tlo{i}",
                )
                for i in range(NLAYERS)
            ]
            tab_hi = [
                dram.tile(
                    [TB, D], bf16, addr_space="Shared",
                    tag=f"thi{i}", name=f"thi{i}",
                )
                for i in range(NLAYERS)
            ]

            def ag_quarter(layer, q):
                if "cc" in ablate:
                    return
                if q == 0 and "cca" in ablate:
                    return
                if q == 1 and "ccb" in ablate:
                    return
                if q == 0:
                    src_ap = ag_a[layer][:]
                    dst_ap = tab_lo[layer][0:TA, :]
                else:
                    src_ap = ag_b[layer][:]
                    dst_ap = tab_hi[layer][0:TB, :]
                nc.gpsimd.collective_compute(
                    "AllGather", mybir.AluOpType.bypass,
                    replica_groups=[list(range(NCORES))],
                    ins=[src_ap], outs=[dst_ap],
                )

            def emit_h_block(b, h_ps, layer):
                """Evacuate one [128 nodes, 128] psum block: relu -> out cols,
                bf16 copy -> ag buffers (except last layer)."""
                r0 = b * 128
                h_f = small.tile([128, D], f32, tag="hf")
                nc.scalar.activation(
                    out=h_f[:], in_=h_ps[:],
                    func=mybir.ActivationFunctionType.Relu,
                )
                if "out" not in ablate:
                    nc.sync.dma_start(
                        out=p_out[r0 : r0 + 128, layer * D : (layer + 1) * D],
                        in_=h_f[:],
                    )
                if layer < NLAYERS:
                    h_b = small.tile([128, D], bf16, tag="hb")
                    nc.scalar.activation(
                        out=h_b[:], in_=h_ps[:],
                        func=mybir.ActivationFunctionType.Relu,
                    )
                    if b < GA:
                        nc.sync.dma_start(
                            out=ag_a[layer][r0 : r0 + 128, :], in_=h_b[:]
                        )
                    else:
                        rb = (b - GA) * 128
                        nc.sync.dma_start(
                            out=ag_b[layer][rb : rb + 128, :], in_=h_b[:]
                        )
                    if b == GA - 1:
                        ag_quarter(layer, 0)
                    elif b == NGRP - 1:
                        ag_quarter(layer, 1)

            def build_S(choff_base, c0, c1):
                """One-hot select matrix for chunk columns [c0, c1)."""
                nch = c1 - c0
                S = sel_pool.tile([128, nch, 128], bf16, tag="sel")
                if "sbuild" in ablate:
                    nc.vector.memset(S[:, 0:1, :], 0)
                else:
                    nc.vector.tensor_tensor(
                        out=S[:],
                        in0=dstrel[:, choff_base + c0 : choff_base + c1][:, :, None]
                        .to_broadcast([128, nch, 128]),
                        in1=iota[:][:, None, :].to_broadcast([128, nch, 128]),
                        op=mybir.AluOpType.is_equal,
                    )
                return S

            # ---- phase 0: h0 = relu(feat @ W0) ----
            for b in range(NGRP):
                h_ps = psC.tile([128, D], f32, space="PSUM", tag="hps")
                nc.tensor.matmul(
                    out=h_ps[:], lhsT=featT[:, b * 128 : (b + 1) * 128],
                    rhs=W0[:], start=True, stop=True,
                )
                emit_h_block(b, h_ps, 0)

            # ---- layers: phase A (lo chunks -> loacc), phase B (hi) ----
            for l in range(NLAYERS):
                loacc = lacc_pool.tile([128, NPCP], bf16, tag="loacc")
                for pi, grp in enumerate(pieces):
                    g0, g1 = grp[0], grp[-1] + 1
                    plo = int(lo_choff[g1] - lo_choff[g0])
                    G_lo = glo_pool.tile([128, plo, D], bf16, tag="glo")
                    if "gather" in ablate:
                        nc.vector.memset(G_lo[:, :, 0:1], 0)
                    else:
                        if l == 0 and pi < 10:
                            # first pool-slot use: zero the tail slots that
                            # truncated gathers leave unwritten (0*NaN poisons
                            # PSUM even under a zero S column)
                            nc.vector.memset(G_lo[:], 0)
                        for gi, g in enumerate(grp):
                            co = int(lo_choff[g] - lo_choff[g0])
                            nw = int(lo_choff[g + 1] - lo_choff[g])
                            nc.gpsimd.dma_gather(
                                G_lo[:, co : co + nw, :], tab_lo[l][:, :],
                                idxlo[:, lo_choff[g] * 8 : lo_choff[g + 1] * 8],
                                int(ncnt_lo[g]), int(ncnt_lo[g]), D,
                                single_packet=False,
                                queue_num=(2 * pi + gi) % 4,
                            )
                    S = build_S(0, int(lo_choff[g0]), int(lo_choff[g1]))
                    for g in grp:
                        acc_ps = psA.tile([128, D], f32, space="PSUM", tag="accT")
                        nmm = NLO[g] if "seg" not in ablate else 1
                        for j in range(nmm):
                            nc.tensor.matmul(
                                out=acc_ps[:],
                                lhsT=G_lo[:, int(lo_choff[g] - lo_choff[g0]) + j, :],
                                rhs=S[:, int(lo_choff[g] - lo_choff[g0]) + j, :],
                                start=(j == 0), stop=(j == nmm - 1),
                            )
                        nc.scalar.activation(
                            out=loacc[:, g * 128 : (g + 1) * 128], in_=acc_ps[:],
                            func=mybir.ActivationFunctionType.Copy,
                        )
                for pi, grp in enumerate(pieces):
                    g0, g1 = grp[0], grp[-1] + 1
                    phi = int(hi_choff[g1] - hi_choff[g0])
                    G_hi = ghi_pool.tile([128, phi, D], bf16, tag="ghi")
                    if "gather" in ablate:
                        nc.vector.memset(G_hi[:, :, 0:1], 0)
                    else:
                        if l == 0 and pi < 10:
                            nc.vector.memset(G_hi[:], 0)
                        for gi, g in enumerate(grp):
                            co = int(hi_choff[g] - hi_choff[g0])
                            nw = int(hi_choff[g + 1] - hi_choff[g])
                            nc.gpsimd.dma_gather(
                                G_hi[:, co : co + nw, :], tab_hi[l][:, :],
                                idxhi[:, hi_choff[g] * 8 : hi_choff[g + 1] * 8],
                                int(ncnt_hi[g]), int(ncnt_hi[g]), D,
                                single_packet=False,
                                queue_num=(2 * pi + gi) % 4,
                            )
                    S = build_S(nch_lo, int(hi_choff[g0]), int(hi_choff[g1]))
                    for g in grp:
                        acc_ps = psA.tile([128, D], f32, space="PSUM", tag="accT")
                        # inject the phase-A partial via an identity matmul so
                        # the add rides the PSUM accumulation chain (PE idle,
                        # DVE loaded)
                        nc.tensor.matmul(
                            out=acc_ps[:], lhsT=eye[:],
                            rhs=loacc[:, g * 128 : (g + 1) * 128],
                            start=True, stop=False,
                        )
                        nmm = NHI[g] if "seg" not in ablate else 1
                        for j in range(nmm):
                            nc.tensor.matmul(
                                out=acc_ps[:],
                                lhsT=G_hi[:, int(hi_choff[g] - hi_choff[g0]) + j, :],
                                rhs=S[:, int(hi_choff[g] - hi_choff[g0]) + j, :],
                                start=False, stop=(j == nmm - 1),
                            )
                        aggT = small.tile([128, D], bf16, tag="aggTsb")
                        nc.vector.tensor_tensor(
                            out=aggT[:], in0=acc_ps[:],
                            in1=invrep[:, g * 128 : (g + 1) * 128],
                            op=mybir.AluOpType.mult,
                        )
                        h_ps = psC.tile([128, D], f32, space="PSUM", tag="hps")
                        nc.tensor.matmul(
                            out=h_ps[:], lhsT=aggT[:], rhs=Ws[:, l, :],
                            start=True, stop=True,
                        )
                        emit_h_block(g, h_ps, l + 1)
    nc.compile()
    return nc


def kernel(feat, src, dst, W0, Ws):
    from concourse.bass_utils import run_bass_kernel_spmd

    plan, slabs = _prepare(feat, src, dst)
    nc = _build(plan)

    W0_np = np.asarray(W0, dtype=np.float32).astype(ml_dtypes.bfloat16)
    Ws_np = (
        np.transpose(np.asarray(Ws, dtype=np.float32), (1, 0, 2))
        .astype(ml_dtypes.bfloat16)
    )  # [fi, layer, fo]
    in_maps = [
        {
            "idx_lo": slabs["idx_lo"][c],
            "idx_hi": slabs["idx_hi"][c],
            "dstrel": slabs["dstrel"][c],
            "invrep": slabs["invrep"][c],
            "featT": slabs["featT"][c],
            "W0": W0_np,
            "Ws": Ws_np,
        }
        for c in range(NCORES)
    ]
    res = None
    last_err = None
    for attempt in range(3):
        try:
            res = run_bass_kernel_spmd(nc, in_maps, core_ids=list(range(NCORES)))
            break
        except Exception as e:  # transient device hiccups (axon RPC, NRT recovery)
            last_err = e
            import time as _time

            _time.sleep(5)
    if res is None:
        raise last_err
    _RUNTIME["nc"] = nc
    _RUNTIME["in_maps"] = in_maps

    pos = slabs["pos"]
    allrows = np.concatenate(
        [res.results[c]["out"] for c in range(NCORES)], axis=0
    )  # [NCORES*NPCP, 512]
    return allrows[pos].astype(np.float32)
